# revision 1
# baseline (speedup 1.0000x reference)
"""Trainium2 Bass kernel for nn_GCN_18820546691816.

The GCN collapses to a per-row MLP chain applied to x1 [B, 112]:
    h1 = relu(x1 @ M1 + b1v)    M1 = kron(A^T, W1)  [112, 56]
    h2 = relu(h1 @ M2 + b2v)    M2 = kron(A^T, W2)  [56, 56]
    h3 = relu(h2 @ Wl1 + bl1)   [56, 24]
    y  = h3 @ Wl2 + bl2         [24, 1]

Device mapping (per core, batch features-on-partitions):
  - All four layer weights occupy disjoint 32x32 subarray regions of the
    128x128 PE array via tile_position, so the four matmuls of a pipelined
    round can run concurrently:
        L1 at rows 0-111,  cols 0-55   (out -> PSUM parts 0-55)
        L2 at rows 0-55,   cols 64-119 (out -> PSUM parts 64-119)
        L3 at rows 64-119, cols 96-119 (out -> PSUM parts 96-119)
        L4 at rows 96-119, cols 64     (out -> PSUM part 64)
  - One ScalarE activation per round does relu+bias for L1 and L2 outputs
    together (disjoint partitions of one PSUM tile).
  - One VectorE tensor_scalar per round does relu+bias for L3 and the final
    +bl2 for L4 (the L4 row uses a -3e38 max-floor so it passes through).
  - x1 is transposed host-side during sharding so tiles load contiguously.
  - Matmuls run in fp16 (1 cycle/col; fp32 runs at 1/4 rate and float32r
    does not support PE subarray tiling). fp16's 10 mantissa bits keep the
    end-to-end error around 1e-3 relative.

Data-parallel over 8 cores: x1T sharded along batch, weights replicated.
"""

from contextlib import ExitStack

import numpy as np

import concourse.bass as bass
import concourse.tile as tile
from concourse import mybir
from concourse.tile_rust import add_dep_helper
from concourse.bass import ds
from concourse.bass_utils import run_bass_kernel_spmd

N_CORES = 8
B = 262144
F_IN = 112
BPC = B // N_CORES        # 32768 samples per core
NB = 1024                 # samples per super-round (2 PSUM banks)
HALF = 512                # matmul free-dim (one PSUM bank, fp32)
T = BPC // NB             # 32 super-rounds of real work
CH = 8                    # s_big output ring depth in rounds

F32 = mybir.dt.float32
F16 = mybir.dt.float16

# fp16 weight blob column layout:
#   [0:56)    M1   (rows 0-111)
#   [56:112)  M2   (rows 0-55)
#   [112:136) Wl1  (rows 64-119)
#   [136:137) Wl2  (rows 96-119)
WGT_COLS = 137
# float32 scalar blob columns: 0 = ACT bias, 1 = DVE bias, 2 = DVE floor
SCL_COLS = 3


def _norm_adj_np(edge_index):
    ei = np.asarray(edge_index)
    src = np.concatenate([ei[0], np.arange(7, dtype=ei.dtype)])
    dst = np.concatenate([ei[1], np.arange(7, dtype=ei.dtype)])
    deg = np.zeros(7, np.float32)
    np.add.at(deg, dst, np.float32(1.0))
    dinv = np.where(deg > 0, deg ** np.float32(-0.5), np.float32(0.0)).astype(
        np.float32
    )
    w = (dinv[src] * dinv[dst]).astype(np.float32)
    A = np.zeros((7, 7), np.float32)
    np.add.at(A, (dst, src), w)
    return A


def _pack_weights(A, W1, W2, Wl1, Wl2):
    M1 = np.kron(A.T, np.asarray(W1)).astype(np.float32)  # [112, 56]
    M2 = np.kron(A.T, np.asarray(W2)).astype(np.float32)  # [56, 56]
    blob = np.zeros((128, WGT_COLS), np.float32)
    blob[0:112, 0:56] = M1
    blob[0:56, 56:112] = M2
    blob[64:120, 112:136] = np.asarray(Wl1, np.float32)
    blob[96:120, 136:137] = np.asarray(Wl2, np.float32)
    return blob.astype(np.float16)


def _pack_scalars(b1, b2, bl1, bl2):
    blob = np.zeros((128, SCL_COLS), np.float32)
    # ACT bias vector: parts 0-55 get b1 (tiled over nodes), 64-119 get b2
    blob[0:56, 0] = np.tile(np.asarray(b1, np.float32), 7)
    blob[64:120, 0] = np.tile(np.asarray(b2, np.float32), 7)
    # DVE scalars for PSUM-B post-op on parts 64-119:
    #   part 64  (L4 out): + bl2, floor -3e38 (no-op relu)
    #   parts 96-119 (L3 out): + bl1, floor 0 (relu)
    blob[64, 1] = np.float32(np.asarray(bl2).reshape(-1)[0])
    blob[96:120, 1] = np.asarray(bl1, np.float32)
    blob[64, 2] = np.float32(-3.0e38)
    return blob


def _split_multiwaits(nc):
    """Walrus accepts only one sync wait per lowered instruction; hoist all
    but the last wait of any multi-wait instruction onto single-wait NOPs
    placed immediately before it on the same engine (engines execute their
    stream in order, so the NOP chain is equivalent)."""
    for f in nc.m.functions:
        for bb in f.blocks:
            out = []
            changed = False
            for inst in bb.instructions:
                si = inst.sync_info
                if si is not None and si.on_wait and len(si.on_wait) > 1:
                    waits = list(si.on_wait)
                    for w in waits[:-1]:
                        nop = mybir.InstNoOp(
                            name=nc.get_next_instruction_name(),
                            engine=inst.engine,
                            sync_info=mybir.SyncInfo(on_wait=[w], on_update=[]),
                            text_hint="split_wait",
                            bass_nofuse=True,
                        )
                        out.append(nop)
                    inst.sync_info = mybir.SyncInfo(
                        on_wait=[waits[-1]], on_update=list(si.on_update or [])
                    )
                    changed = True
                out.append(inst)
            if changed:
                bb.instructions = out


def _build_nc():
    nc = bass.Bass("TRN2", target_bir_lowering=False, debug=False)
    xT = nc.dram_tensor("xT", [F_IN, BPC], F16, kind="ExternalInput").ap()
    wgt = nc.dram_tensor("wgt", [128, WGT_COLS], F16, kind="ExternalInput").ap()
    scl = nc.dram_tensor("scl", [128, SCL_COLS], F32, kind="ExternalInput").ap()
    # One output tensor per 8-round window cycle: separate tensors so the
    # final DMAs carry no WAW chain (walrus allows one sync wait per
    # instruction, and a DMA-completion wait must be the only one). Window w
    # of chunk k holds block 8k + ((w - 5) % 8); the host undoes the
    # permutation.
    ys = [
        nc.dram_tensor(f"y{k}", [1, CH * NB], F16, kind="ExternalOutput").ap()
        for k in range(T // CH)
    ]

    with tile.TileContext(nc) as tc, ExitStack() as ctx:
        wpool = ctx.enter_context(tc.tile_pool(name="wpool", bufs=1))
        # One slot per xt tile: no slot reuse means the xt DMAs carry no
        # WAR/WAW semaphore waits at all. 32 x 4KB/partition = 16 MB of SBUF.
        xpool = ctx.enter_context(tc.tile_pool(name="xpool", bufs=T))
        hpool = ctx.enter_context(tc.tile_pool(name="hpool", bufs=4))
        # Persistent ping-pong PSUM tiles (not pool-rotated): slot releases
        # are what force un-elidable PE self-waits on the first writer of a
        # reused slot, and plain same-tile WAW on one engine needs no sem.
        ps_pool = ctx.enter_context(tc.tile_pool(name="ps", bufs=1, space="PSUM"))

        wb = wpool.tile([128, WGT_COLS], F16)
        nc.sync.dma_start(wb[:, :], wgt)
        sb = wpool.tile([128, SCL_COLS], F32)
        nc.sync.dma_start(sb[:, :], scl)
        w1 = wb[0:112, 0:56]
        w2 = wb[0:56, 56:112]
        w3 = wb[64:120, 112:136]
        w4 = wb[96:120, 136:137]
        actbias = sb[0:120, 0]
        sbias = sb[64:120, 1]
        sfloor = sb[64:120, 2]

        relu = mybir.ActivationFunctionType.Relu
        add_op = mybir.AluOpType.add
        max_op = mybir.AluOpType.max

        xt = {}   # t -> xT tile [112, NB] f32r
        h = {}    # t -> h tile [128, NB] f32r: [0:56]=h1(t), [64:120]=h2(t-2)
        pA_pp = [ps_pool.tile([128, NB], F32, name=f"pApp{i}", tag=f"pA{i}")
                 for i in range(2)]
        pB_pp = [ps_pool.tile([128, NB], F32, name=f"pBpp{i}", tag=f"pB{i}")
                 for i in range(2)]

        # s ring: round t uses column window t % CH. Partition 64 of window
        # w(t) = y(t-5); partitions 96-119 = h3(t-3). One gpsimd DMA per CH
        # rounds ships the whole partition-64 row.
        s_big = wpool.tile([128, CH * NB], F16)

        def s_win(t):
            return s_big[:, ds((t % CH) * NB, NB)]

        # Engine "clock pumps": walrus accepts a single sync wait per
        # instruction, and engines do not observe their own semaphore ticks.
        # A 1x1 op at the end of each round waits on its own engine's
        # previous pump tick, which (a) is itself a legal single wait and
        # (b) advances the engine's observed self-tick past every
        # same-engine hazard from earlier rounds, so the real instructions
        # carry only their single cross-engine data wait.
        act_scr = wpool.tile([1, 1], F32)
        dve_scr = wpool.tile([1, 1], F32)
        # preamble: absorb the wgt/scl DMA lane ticks per engine
        nc.tensor.matmul(pA_pp[0][96:97, 0:1], wb[0:1, 0:1], wb[0:1, 0:1],
                         start=True, stop=True, tile_position=(0, 96))
        nc.scalar.copy(act_scr[0:1, 0:1], sb[0:1, 0:1])
        nc.vector.tensor_copy(dve_scr[0:1, 0:1], sb[0:1, 1:2])

        # Pipeline lags: L1 block t at round t; L2 at t+2; L3 at t+3; L4 at
        # t+5. Every PE instruction reads data produced >= 1 round earlier,
        # so the PE never stalls on the current round's ACT/DVE. Emission
        # order per round keeps each matmul at one new semaphore wait.
        for t in range(T + 5):
            if t < T:
                xt[t] = xpool.tile([F_IN, NB], F16, name=f"xt{t}", tag="xt")
                nc.sync.dma_start(xt[t][:, :], xT[:, ds(t * NB, NB)])
            pA = pA_pp[t % 2]
            pB = pB_pp[t % 2]

            last_pe = last_act = last_dve = None

            if t >= 13 and (t - 13) % CH == 0:
                # absorb the latest out-DMA lane tick into the DVE clock
                # (window 0 already shipped; safe to scribble)
                last_dve = nc.vector.memset(s_big[64:65, 0:1], 0.0)

            for j in range(NB // HALF):
                c = ds(j * HALF, HALF)
                if 2 <= t <= T + 1:  # L2(t-2)
                    last_pe = nc.tensor.matmul(
                        pA[64:120, c], w2, h[t - 2][0:56, c],
                        start=True, stop=True, tile_position=(0, 64),
                    )
            h.pop(t - 2, None)
            for j in range(NB // HALF):
                c = ds(j * HALF, HALF)
                if t < T:  # L1(t)
                    last_pe = nc.tensor.matmul(
                        pA[0:56, c], w1, xt[t][:, c],
                        start=True, stop=True, tile_position=(0, 0),
                    )
            xt.pop(t, None)
            for j in range(NB // HALF):
                c = ds(j * HALF, HALF)
                if 5 <= t:  # L4(t-5)
                    last_pe = nc.tensor.matmul(
                        pB[64:65, c], w4, s_win(t - 2)[96:120, c],
                        start=True, stop=True, tile_position=(96, 64),
                    )
            for j in range(NB // HALF):
                c = ds(j * HALF, HALF)
                if 3 <= t <= T + 2:  # L3(t-3)
                    last_pe = nc.tensor.matmul(
                        pB[96:120, c], w3, h[t - 1][64:120, c],
                        start=True, stop=True, tile_position=(64, 96),
                    )

            if t <= T + 1:
                h[t] = hpool.tile([128, NB], F16, name=f"h{t}", tag="h")
                last_act = nc.scalar.activation(
                    h[t][0:120, :], pA[0:120, :], relu, bias=actbias[:, None]
                )
            if t >= 3:
                last_dve = nc.vector.tensor_scalar(
                    s_win(t)[64:120, :], pB[64:120, :],
                    sbias[:, None], sfloor[:, None], add_op, max_op,
                )
            if t >= 12 and (t - 12) % CH == 0:
                nc.gpsimd.dma_start(ys[(t - 12) // CH][:, :], s_big[64:65, :])

            # End-of-round engine clock pumps, order-pinned (sync=False)
            # behind the round's last real op so the scheduler cannot hoist
            # them. Each pump's only semaphore wait is its own engine's
            # previous pump tick, which advances the engine's observed
            # self-clock past every same-engine hazard from earlier rounds.
            if last_act is not None:
                p = nc.scalar.copy(act_scr[0:1, 0:1], act_scr[0:1, 0:1])
                add_dep_helper(p.ins, last_act.ins, sync=False, reason="pin act pump")
            if last_dve is not None:
                p = nc.vector.tensor_copy(dve_scr[0:1, 0:1], dve_scr[0:1, 0:1])
                add_dep_helper(p.ins, last_dve.ins, sync=False, reason="pin dve pump")



    _split_multiwaits(nc)
    return nc


_NC_CACHE = None


def _get_nc():
    global _NC_CACHE
    if _NC_CACHE is None:
        _NC_CACHE = _build_nc()
    return _NC_CACHE


def _make_in_maps(x1, edge_index, W1, b1, W2, b2, Wl1, bl1, Wl2, bl2):
    x1 = np.asarray(x1, np.float32)
    A = _norm_adj_np(edge_index)
    wgt = _pack_weights(A, W1, W2, Wl1, Wl2)
    scl = _pack_scalars(b1, b2, bl1, bl2)
    x1T = np.ascontiguousarray(x1.T.astype(np.float16))  # [112, B] fp16
    return [
        {
            "xT": np.ascontiguousarray(x1T[:, c * BPC : (c + 1) * BPC]),
            "wgt": wgt,
            "scl": scl,
        }
        for c in range(N_CORES)
    ]


def kernel(x1, edge_index, W1, b1, W2, b2, Wl1, bl1, Wl2, bl2, **_unused):
    in_maps = _make_in_maps(x1, edge_index, W1, b1, W2, b2, Wl1, bl1, Wl2, bl2)
    nc = _get_nc()
    res = run_bass_kernel_spmd(nc, in_maps, list(range(N_CORES)))
    return _gather_y(res.results)


def _gather_y(results):
    # window w of chunk k holds block 8k + ((w - 5) % 8): block b sits at
    # window (b + 5) % 8
    worder = [(b + 5) % CH for b in range(CH)]
    parts = []
    for c in range(N_CORES):
        for k in range(T // CH):
            yk = results[c][f"y{k}"].reshape(CH, NB)
            parts.append(yk[worder].reshape(-1))
    return np.concatenate(parts).reshape(B, 1).astype(np.float32)

